# revision 51
# baseline (speedup 1.0000x reference)
"""Trainium2 Bass kernel for nn_ContrastiveLoss (B=4096, D=1024, 8 cores).

loss = mean over [B,B] of, for each sim in (sim0, sim1):
    L*(1-sim) + (1-L)*relu(sim-0.5)
where sim_k = cos_sim(fc_feats_k[i], textual_features[j]).

Per-sim element identity (r = relu(s - m)):
    L*(1-s) + (1-L)*r  =  L + r - L*(s + r)

Strategy (data-parallel i-rows; t replicated -- measured AllGather
latency on this fabric is ~130us for 4MB, far too slow to use):
  * Each core gets its 512-row slice of fc_feats_0/1, its labels slice
    TRANSPOSED on host ([B, 512], pure layout), and the FULL
    textual_features transposed on host ([D, B], pure layout).
  * Phase B matmuls put t on the STATIONARY side: out psum is
    [128 j-part, 512 i-free], so the per-j t-norm is a per-PARTITION
    scalar -- no elementwise renormalization pass over t at all.
    t is cast raw to fp8 (values ~N(0,1), in e4m3 range).
  * t column norms: ACT squares (fp8) + fp8 ones-matvec accumulated
    over d in PSUM [1,512], sqrt, tiny PE transposes into [j-part,1]
    vectors, then beta = 1/(64*||t_j||) and c = 32*||t_j|| as [128,1]
    scalars per j-tile.
  * One custom DVE instruction per psum tile computes (u = raw psum):
        q = beta_j*(ru - L*(u + ru)) + L,   ru = relu(u - c_j)
    with free-dim accumulation; summing q over everything equals the
    sum of loss elements exactly (s = beta*u, r = beta*ru).
  * f0/f1: row-normalized (x64) to fp8 and PE-transposed as usual;
    they are the MOVING operand [d, 512 i].
  * DMA: sync queue (engine has no compute, may block freely) carries
    all 16MB of t; the gpsimd SW queue carries f0/f1 then labels (f32).
    Compute engines never issue DMAs -- a blocked DMA issue stalls the
    whole engine stream.
  * Host sums the 8x[128] partials and divides by B*B.

Self-contained: hardcodes shapes; only needs the concourse package.
"""

import os
import sys

import numpy as np

B = 4096
D = 1024
NCORES = 8
ROWS = B // NCORES          # 512 i-rows of f0/f1 per core
IT = ROWS // 128            # 4 f i-tiles per core
KS = D // 128               # 8 k-subtiles (contraction)
JTILES = B // 128           # 32 j-tiles
MARGIN = 0.5

JC = int(os.environ.get("KERNEL_TCHUNKS", "4"))  # t j-chunks
CW = B // JC                # columns per chunk (1024)
WPC = CW // 512             # 512-wide matvec windows per chunk
JPC = CW // 128             # j-tiles per chunk (8)
NSLOT = JTILES * 2          # accumulation slots (x2 sims) = 64

_CACHE = {}


def _import_concourse():
    try:
        import concourse.bass  # noqa: F401
    except ImportError:
        for p in ("/opt/trn_rl_repo", "/root/.axon_site/_ro/trn_rl_repo"):
            if os.path.isdir(p) and p not in sys.path:
                sys.path.insert(0, p)
        import concourse.bass  # noqa: F401


def _register_fused_op():
    """Register the fused contrastive-loss DVE op (idempotent).

    body = (r - Src1*(Src0 + r))*C0 + Src1, r = relu(Src0 - C1); accum=add.
    s0 (C0) and s1 (C1) are per-partition [128,1] APs at the call site.
    """
    from operator import add

    import concourse.dve_ops as dve_ops
    from concourse.dve_spec import C0, C1, Spec, Src0, Src1, Zero, lower, relu
    from concourse.dve_uop import DveOpSpec

    name = "CL_FUSED_Q_ANT"
    if name in dve_ops._SUB_OPCODE_FOR_NAME:
        return next(op for op in dve_ops.OPS if op.name == name)

    r_ = relu(Src0 - C1)
    body = (r_ - Src1 * (Src0 + r_)) * C0 + Src1

    def _ref(in0, in1, c0, c1, c2):
        x = in0.astype(np.float32)
        L = in1.astype(np.float32)
        r = np.maximum(np.nan_to_num(x - c1, nan=0.0), 0.0)
        b = ((r - L * (x + r)) * c0 + L).astype(np.float32)
        return b, b.reshape(b.shape[0], -1).sum(axis=-1, keepdims=True)

    spec = Spec(body=body, accum=add, accum_init=Zero, reference=_ref)
    row = max(dve_ops._SUB_OPCODE_FOR_NAME.values()) + 1
    assert row < 0x20
    dve_ops._SUB_OPCODE_FOR_NAME[name] = row
    shas = {}
    for ver in ("v3", "v4"):
        try:
            uops = lower(spec, ver=ver)
        except Exception:
            continue
        shas[ver] = DveOpSpec(
            name=name, opcode=row, uops=uops, rd1_en=True).sha(ver)
    op = dve_ops.DveOp(name, spec, False, shas)
    dve_ops.OPS.append(op)
    dve_ops.CUSTOM_DVE_SPECS[name] = spec
    return op


def _build_nc():
    """Build + schedule + compile the per-core Bass program (SPMD)."""
    _import_concourse()
    import concourse.mybir as mybir
    import concourse.tile as tile
    from concourse import bacc
    from concourse.masks import make_identity

    fused_op = _register_fused_op()

    f32 = mybir.dt.float32
    bf16 = mybir.dt.bfloat16
    wdt = mybir.dt.float8e4
    AF = mybir.ActivationFunctionType
    OP = mybir.AluOpType
    AX = mybir.AxisListType
    DR = mybir.MatmulPerfMode.DoubleRow

    nc = bacc.Bacc(
        "TRN2",
        target_bir_lowering=False,
        debug=False,
        num_devices=NCORES,
    )

    f0_d = nc.dram_tensor("f0", [ROWS, D], f32, kind="ExternalInput").ap()
    f1_d = nc.dram_tensor("f1", [ROWS, D], f32, kind="ExternalInput").ap()
    txT_d = nc.dram_tensor("txT", [D, B], f32, kind="ExternalInput").ap()
    labT_d = nc.dram_tensor("labT", [B, ROWS], f32,
                            kind="ExternalInput").ap()
    out_d = nc.dram_tensor("outv", [128, 1], f32, kind="ExternalOutput").ap()

    with tile.TileContext(nc) as tc:
        with (
            tc.tile_pool(name="constp", bufs=1) as constp,
            tc.tile_pool(name="traw", bufs=8) as trawp,
            tc.tile_pool(name="trawo", bufs=16) as trawop,
            tc.tile_pool(name="fnat", bufs=4) as fnatp,
            tc.tile_pool(name="stage", bufs=2) as stage,
            tc.tile_pool(name="small", bufs=8) as small,
            tc.tile_pool(name="nrm", bufs=1) as nrmp,
            tc.tile_pool(name="wT", bufs=1) as wTp,
            tc.tile_pool(name="tnp", bufs=1) as tnp,
            tc.tile_pool(name="labp", bufs=1) as labp,
            tc.tile_pool(name="scrp", bufs=2) as scrp,
            tc.tile_pool(name="accp", bufs=1) as accp,
            tc.tile_pool(name="tpsum", bufs=2, space="PSUM") as tpsum,
            tc.tile_pool(name="gpsum", bufs=2, space="PSUM") as gpsum,
            tc.tile_pool(name="mpsum", bufs=4, space="PSUM") as mpsum,
        ):
            ident = constp.tile([128, 128], bf16)
            make_identity(nc, ident)

            qacc = accp.tile([128, NSLOT], f32)

            f0T = wTp.tile([128, KS, ROWS], wdt)   # moving operands (x64)
            f1T = wTp.tile([128, KS, ROWS], wdt)
            tnc = tnp.tile([128, KS, B], wdt)      # raw fp8 t (stationary)
            LtT = labp.tile([128, JTILES, ROWS], bf16)   # labels^T
            # per-j-tile norm-derived scalars
            nrmT = nrmp.tile([128, JTILES], f32)   # ||t_j||
            beta = nrmp.tile([128, JTILES], f32)   # 1/(64 ||t_j||)
            cthr = nrmp.tile([128, JTILES], f32)   # 64*MARGIN*||t_j||

            # ---- input DMAs on parallel queues ----
            # sync queue: t even-ks halves interleaved with label groups
            # scalar queue: chunk0 odd half, f0/f1, remaining odd halves
            labT3 = labT_d.rearrange("(g p) i -> p g i", p=128)
            traws = {}
            # scalar hw queue: odd-ks t tiles. Fully-resident buffers
            # (bufs=16, no reuse) so these issues NEVER block the ACT
            # engine's compute stream; emitted first, in one burst.
            for jc in range(JC):
                for ks in range(1, KS, 2):
                    tr = trawop.tile([128, CW], f32, tag="trawo",
                                     name=f"trawo_{jc}_{ks}")
                    nc.scalar.dma_start(
                        tr, txT_d[ks * 128:(ks + 1) * 128,
                                  jc * CW:(jc + 1) * CW])
                    traws[(jc, ks)] = tr
            # sync queue (no compute, may block freely): even-ks t tiles
            for jc in range(JC):
                for ks in range(0, KS, 2):
                    tr = trawp.tile([128, CW], f32, tag="traw",
                                    name=f"traw_{jc}_{ks}")
                    nc.sync.dma_start(
                        tr, txT_d[ks * 128:(ks + 1) * 128,
                                  jc * CW:(jc + 1) * CW])
                    traws[(jc, ks)] = tr
            # gpsimd SW queue: f first, then labels (f32->bf16 cast DMA)
            fnats = []
            for fi, src in enumerate((f0_d, f1_d)):
                for it in range(IT):
                    nat = fnatp.tile([128, D], f32, tag="fnat",
                                     name=f"fnat_{fi}_{it}")
                    nc.gpsimd.dma_start(nat, src[it * 128:(it + 1) * 128, :])
                    fnats.append(nat)
            for jg in range(4):
                nc.gpsimd.dma_start(
                    LtT[:, jg * 8:(jg + 1) * 8, :],
                    labT3[:, jg * 8:(jg + 1) * 8, :],
                )

            def chunk_prep(jc):
                """fp8 casts for t-chunk jc (ACT only; no DMA issuers)."""
                c0 = jc * CW
                for ks in range(KS):
                    nc.scalar.copy(tnc[:, ks, c0:c0 + CW], traws[(jc, ks)])

            def f_path():
                for it in range(IT):
                    for fi, fT in enumerate((f0T, f1T)):
                        nat = fnats[fi * IT + it]
                        key = f"f{fi}_{it}"
                        sq = stage.tile([128, D], bf16, tag="fsq",
                                        name=f"fsq_{key}")
                        ssq = small.tile([128, 1], f32, tag="ssq",
                                         name=f"ssq_{key}")
                        nc.scalar.activation(sq, nat, AF.Square,
                                             accum_out=ssq)
                        nrm = small.tile([128, 1], f32, tag="nrm",
                                         name=f"nrm_{key}")
                        nc.scalar.activation(nrm, ssq, AF.Sqrt,
                                             scale=1.0 / 4096.0)
                        rin = small.tile([128, 1], f32, tag="rin",
                                         name=f"rin_{key}")
                        nc.vector.reciprocal(rin, nrm)   # 64 / ||f_i||
                        nrmd = stage.tile([128, D], bf16, tag="nrmd",
                                          name=f"nrmd_{key}")
                        nc.vector.tensor_scalar_mul(nrmd, nat, rin)
                        for ks in range(KS):
                            pst = tpsum.tile([128, 128], bf16, tag="pst",
                                             name=f"pst_{key}_{ks}")
                            nc.tensor.transpose(
                                pst, nrmd[:, ks * 128:(ks + 1) * 128], ident)
                            dst = fT[:, ks, it * 128:(it + 1) * 128]
                            nc.scalar.copy(dst, pst)

            def phase_B(jc):
                for b in range(JPC):
                    jt = jc * JPC + b
                    jsl1 = slice(jt, jt + 1)
                    # norm^2 = diag(T_jt^T T_jt) via a small Gram matmul,
                    # diag extracted by a gpsimd identity-mask STT w/ accum
                    gp = gpsum.tile([128, 128], f32, tag="gp",
                                    name=f"gp_{jt}")
                    jb = slice(jt * 128, (jt + 1) * 128)
                    for k2 in range(KS // 2):
                        ksl = slice(2 * k2, 2 * k2 + 2)
                        nc.tensor.matmul(
                            gp, tnc[:, ksl, jb], tnc[:, ksl, jb],
                            perf_mode=DR,
                            start=(k2 == 0), stop=(k2 == KS // 2 - 1),
                        )
                    gsc = scrp.tile([128, 128], bf16, tag="gsc",
                                    name=f"gsc_{jt}")
                    nc.vector.scalar_tensor_tensor(
                        out=gsc, in0=gp, scalar=1.0, in1=ident,
                        op0=OP.mult, op1=OP.mult,
                        accum_out=nrmT[:, jsl1])
                    # nrm64 = 64*||t_j||; beta = 1/nrm64; cthr = margin*nrm64
                    nc.scalar.activation(cthr[:, jsl1], nrmT[:, jsl1],
                                         AF.Sqrt, scale=4096.0)
                    nc.vector.reciprocal(beta[:, jsl1], cthr[:, jsl1])
                    nc.vector.tensor_scalar_mul(cthr[:, jsl1],
                                                cthr[:, jsl1], MARGIN)
                    for sim, fT in enumerate((f0T, f1T)):
                        ps = mpsum.tile([128, ROWS], f32, tag="ps",
                                        name=f"ps_{jt}_{sim}")
                        for k2 in range(KS // 2):
                            ksl = slice(2 * k2, 2 * k2 + 2)
                            nc.tensor.matmul(
                                ps,
                                tnc[:, ksl, jt * 128:(jt + 1) * 128],
                                fT[:, ksl, :],
                                perf_mode=DR,
                                start=(k2 == 0), stop=(k2 == KS // 2 - 1),
                            )
                        slot = jt * 2 + sim
                        scr = scrp.tile([128, ROWS], bf16, tag="scr",
                                        name=f"scr_{slot}")
                        nc.vector._custom_dve(
                            fused_op,
                            out=scr,
                            in0=ps,
                            in1=LtT[:, jt, :],
                            s0=beta[:, jsl1],
                            s1=cthr[:, jsl1],
                            imm2=0.0,
                            accum_out=qacc[:, slot:slot + 1],
                        )

            # emission order keeps each in-order engine stream aligned
            # with data arrival: prep(0), f, B(0), prep(1), B(1), ...
            chunk_prep(0)
            f_path()
            for jc in range(JC):
                phase_B(jc)
                if jc + 1 < JC:
                    chunk_prep(jc + 1)

            # ---- finisher ----
            ov = small.tile([128, 1], f32, tag="fin", name="ov")
            nc.vector.reduce_sum(ov, qacc, axis=AX.X)
            nc.sync.dma_start(out_d, ov)

    nc.compile()
    return nc


def _get_nc():
    if "nc" not in _CACHE:
        _CACHE["nc"] = _build_nc()
    return _CACHE["nc"]


def _make_in_maps(fc_feats_0, fc_feats_1, textual_features, labels):
    txT = np.ascontiguousarray(
        np.asarray(textual_features, dtype=np.float32).T)
    labels = np.asarray(labels, dtype=np.float32)
    in_maps = []
    for c in range(NCORES):
        sl = slice(c * ROWS, (c + 1) * ROWS)
        in_maps.append({
            "f0": np.ascontiguousarray(fc_feats_0[sl], dtype=np.float32),
            "f1": np.ascontiguousarray(fc_feats_1[sl], dtype=np.float32),
            "txT": txT,
            "labT": np.ascontiguousarray(labels[sl].T),
        })
    return in_maps


def run(fc_feats_0, fc_feats_1, textual_features, labels, trace=False):
    """Run on 8 NeuronCores; returns (loss_scalar, BassKernelResults)."""
    _import_concourse()
    from concourse.bass_utils import run_bass_kernel_spmd

    nc = _get_nc()
    in_maps = _make_in_maps(np.asarray(fc_feats_0), np.asarray(fc_feats_1),
                            np.asarray(textual_features), np.asarray(labels))
    res = run_bass_kernel_spmd(nc, in_maps, list(range(NCORES)), trace=trace)
    total = 0.0
    for c in range(NCORES):
        total += float(np.asarray(res.results[c]["outv"],
                                  dtype=np.float64).sum())
    loss = total / float(B * B)
    return np.asarray(loss, dtype=np.float32), res


def kernel(fc_feats_0, fc_feats_1, textual_features, labels):
    loss, _ = run(fc_feats_0, fc_feats_1, textual_features, labels,
                  trace=False)
    return loss


# revision 54
# speedup vs baseline: 1.2443x; 1.2443x over previous
"""Trainium2 Bass kernel for nn_ContrastiveLoss (B=4096, D=1024, 8 cores).

loss = mean over [B,B] of, for each sim in (sim0, sim1):
    L*(1-sim) + (1-L)*relu(sim-0.5)
where sim_k = cos_sim(fc_feats_k[i], textual_features[j]).

Per-sim element identity (r = relu(s - m)):
    L*(1-s) + (1-L)*r  =  L + r - L*(s + r)

Strategy (data-parallel i-rows; t replicated -- measured AllGather
latency on this fabric is ~130us for 4MB, far too slow to use):
  * Each core gets its 512-row slice of fc_feats_0/1, its labels slice
    TRANSPOSED on host ([B, 512], pure layout), and the FULL
    textual_features transposed on host ([D, B], pure layout).
  * Phase B matmuls put t on the STATIONARY side: out psum is
    [128 j-part, 512 i-free], so the per-j t-norm is a per-PARTITION
    scalar -- no elementwise renormalization pass over t at all.
    t is cast raw to fp8 (values ~N(0,1), in e4m3 range).
  * t column norms: ACT squares (fp8) + fp8 ones-matvec accumulated
    over d in PSUM [1,512], sqrt, tiny PE transposes into [j-part,1]
    vectors, then beta = 1/(64*||t_j||) and c = 32*||t_j|| as [128,1]
    scalars per j-tile.
  * One custom DVE instruction per psum tile computes (u = raw psum):
        q = beta_j*(ru - L*(u + ru)) + L,   ru = relu(u - c_j)
    with free-dim accumulation; summing q over everything equals the
    sum of loss elements exactly (s = beta*u, r = beta*ru).
  * f0/f1: row-normalized (x64) to fp8 and PE-transposed as usual;
    they are the MOVING operand [d, 512 i].
  * DMA: sync queue (engine has no compute, may block freely) carries
    all 16MB of t; the gpsimd SW queue carries f0/f1 then labels (f32).
    Compute engines never issue DMAs -- a blocked DMA issue stalls the
    whole engine stream.
  * Host sums the 8x[128] partials and divides by B*B.

Self-contained: hardcodes shapes; only needs the concourse package.
"""

import os
import sys

import numpy as np

B = 4096
D = 1024
NCORES = 8
ROWS = B // NCORES          # 512 i-rows of f0/f1 per core
IT = ROWS // 128            # 4 f i-tiles per core
KS = D // 128               # 8 k-subtiles (contraction)
JTILES = B // 128           # 32 j-tiles
MARGIN = 0.5

JC = int(os.environ.get("KERNEL_TCHUNKS", "4"))  # t j-chunks
CW = B // JC                # columns per chunk (1024)
WPC = CW // 512             # 512-wide matvec windows per chunk
JPC = CW // 128             # j-tiles per chunk (8)
NSLOT = JTILES * 2          # accumulation slots (x2 sims) = 64

_CACHE = {}


def _import_concourse():
    try:
        import concourse.bass  # noqa: F401
    except ImportError:
        for p in ("/opt/trn_rl_repo", "/root/.axon_site/_ro/trn_rl_repo"):
            if os.path.isdir(p) and p not in sys.path:
                sys.path.insert(0, p)
        import concourse.bass  # noqa: F401


def _register_fused_op():
    """Register the fused contrastive-loss DVE op (idempotent).

    body = (r - Src1*(Src0 + r))*C0 + Src1, r = relu(Src0 - C1); accum=add.
    s0 (C0) and s1 (C1) are per-partition [128,1] APs at the call site.
    """
    from operator import add

    import concourse.dve_ops as dve_ops
    from concourse.dve_spec import C0, C1, Spec, Src0, Src1, Zero, lower, relu
    from concourse.dve_uop import DveOpSpec

    name = "CL_FUSED_Q_ANT"
    if name in dve_ops._SUB_OPCODE_FOR_NAME:
        return next(op for op in dve_ops.OPS if op.name == name)

    r_ = relu(Src0 - C1)
    body = (r_ - Src1 * (Src0 + r_)) * C0 + Src1

    def _ref(in0, in1, c0, c1, c2):
        x = in0.astype(np.float32)
        L = in1.astype(np.float32)
        r = np.maximum(np.nan_to_num(x - c1, nan=0.0), 0.0)
        b = ((r - L * (x + r)) * c0 + L).astype(np.float32)
        return b, b.reshape(b.shape[0], -1).sum(axis=-1, keepdims=True)

    spec = Spec(body=body, accum=add, accum_init=Zero, reference=_ref)
    row = max(dve_ops._SUB_OPCODE_FOR_NAME.values()) + 1
    assert row < 0x20
    dve_ops._SUB_OPCODE_FOR_NAME[name] = row
    shas = {}
    for ver in ("v3", "v4"):
        try:
            uops = lower(spec, ver=ver)
        except Exception:
            continue
        shas[ver] = DveOpSpec(
            name=name, opcode=row, uops=uops, rd1_en=True).sha(ver)
    op = dve_ops.DveOp(name, spec, False, shas)
    dve_ops.OPS.append(op)
    dve_ops.CUSTOM_DVE_SPECS[name] = spec
    return op


def _build_nc():
    """Build + schedule + compile the per-core Bass program (SPMD)."""
    _import_concourse()
    import concourse.mybir as mybir
    import concourse.tile as tile
    from concourse import bacc
    from concourse.masks import make_identity

    fused_op = _register_fused_op()

    f32 = mybir.dt.float32
    bf16 = mybir.dt.bfloat16
    wdt = mybir.dt.float8e4
    AF = mybir.ActivationFunctionType
    OP = mybir.AluOpType
    AX = mybir.AxisListType
    DR = mybir.MatmulPerfMode.DoubleRow

    nc = bacc.Bacc(
        "TRN2",
        target_bir_lowering=False,
        debug=False,
        num_devices=NCORES,
    )

    f0_d = nc.dram_tensor("f0", [ROWS, D], f32, kind="ExternalInput").ap()
    f1_d = nc.dram_tensor("f1", [ROWS, D], f32, kind="ExternalInput").ap()
    txT_d = nc.dram_tensor("txT", [D, B], f32, kind="ExternalInput").ap()
    labT_d = nc.dram_tensor("labT", [B, ROWS], f32,
                            kind="ExternalInput").ap()
    out_d = nc.dram_tensor("outv", [128, 1], f32, kind="ExternalOutput").ap()

    with tile.TileContext(nc) as tc:
        with (
            tc.tile_pool(name="constp", bufs=1) as constp,
            tc.tile_pool(name="traw", bufs=12) as trawp,
            tc.tile_pool(name="fnat", bufs=4) as fnatp,
            tc.tile_pool(name="stage", bufs=3) as stage,
            tc.tile_pool(name="small", bufs=8) as small,
            tc.tile_pool(name="nrm", bufs=1) as nrmp,
            tc.tile_pool(name="wT", bufs=1) as wTp,
            tc.tile_pool(name="tnp", bufs=1) as tnp,
            tc.tile_pool(name="labp", bufs=1) as labp,
            tc.tile_pool(name="scrp", bufs=2) as scrp,
            tc.tile_pool(name="accp", bufs=1) as accp,
            tc.tile_pool(name="tpsum", bufs=2, space="PSUM") as tpsum,
            tc.tile_pool(name="gpsum", bufs=2, space="PSUM") as gpsum,
            tc.tile_pool(name="mpsum", bufs=4, space="PSUM") as mpsum,
        ):
            ident = constp.tile([128, 128], bf16)
            make_identity(nc, ident)

            qacc = accp.tile([128, NSLOT], f32)

            f0T = wTp.tile([128, KS, ROWS], wdt)   # moving operands (x64)
            f1T = wTp.tile([128, KS, ROWS], wdt)
            tnc = tnp.tile([128, KS, B], wdt)      # raw fp8 t (stationary)
            LtT = labp.tile([128, JTILES, ROWS], f32)    # labels^T
            # per-j-tile norm-derived scalars
            nrmT = nrmp.tile([128, JTILES], f32)   # ||t_j||
            beta = nrmp.tile([128, JTILES], f32)   # 1/(64 ||t_j||)
            cthr = nrmp.tile([128, JTILES], f32)   # 64*MARGIN*||t_j||

            # ---- input DMAs on parallel queues ----
            # sync queue: t even-ks halves interleaved with label groups
            # scalar queue: chunk0 odd half, f0/f1, remaining odd halves
            labT3 = labT_d.rearrange("(g p) i -> p g i", p=128)
            traws = {}
            # sync queue (no compute on that engine): ALL t tiles
            for jc in range(JC):
                for ks in range(KS):
                    tr = trawp.tile([128, CW], f32, tag="traw",
                                    name=f"traw_{jc}_{ks}")
                    nc.sync.dma_start(
                        tr, txT_d[ks * 128:(ks + 1) * 128,
                                  jc * CW:(jc + 1) * CW])
                    traws[(jc, ks)] = tr
            # gpsimd SW queue: f first, then labels (f32)
            fnats = []
            for fi, src in enumerate((f0_d, f1_d)):
                for it in range(IT):
                    nat = fnatp.tile([128, D], f32, tag="fnat",
                                     name=f"fnat_{fi}_{it}")
                    nc.gpsimd.dma_start(nat, src[it * 128:(it + 1) * 128, :])
                    fnats.append(nat)
            for jg in range(4):
                nc.gpsimd.dma_start(
                    LtT[:, jg * 8:(jg + 1) * 8, :],
                    labT3[:, jg * 8:(jg + 1) * 8, :],
                )

            def chunk_prep(jc):
                """fp8 casts for t-chunk jc (ACT only; no DMA issuers)."""
                c0 = jc * CW
                for ks in range(KS):
                    nc.scalar.copy(tnc[:, ks, c0:c0 + CW], traws[(jc, ks)])

            def f_path():
                for it in range(IT):
                    for fi, fT in enumerate((f0T, f1T)):
                        nat = fnats[fi * IT + it]
                        key = f"f{fi}_{it}"
                        sq = stage.tile([128, D], bf16, tag="fsq",
                                        name=f"fsq_{key}")
                        ssq = small.tile([128, 1], f32, tag="ssq",
                                         name=f"ssq_{key}")
                        nc.scalar.activation(sq, nat, AF.Square,
                                             accum_out=ssq)
                        nrm = small.tile([128, 1], f32, tag="nrm",
                                         name=f"nrm_{key}")
                        nc.scalar.activation(nrm, ssq, AF.Sqrt,
                                             scale=1.0 / 4096.0)
                        rin = small.tile([128, 1], f32, tag="rin",
                                         name=f"rin_{key}")
                        nc.vector.reciprocal(rin, nrm)   # 64 / ||f_i||
                        nrmd = stage.tile([128, D], bf16, tag="nrmd",
                                          name=f"nrmd_{key}")
                        nc.vector.tensor_scalar_mul(nrmd, nat, rin)
                        for ks in range(KS):
                            pst = tpsum.tile([128, 128], bf16, tag="pst",
                                             name=f"pst_{key}_{ks}")
                            nc.tensor.transpose(
                                pst, nrmd[:, ks * 128:(ks + 1) * 128], ident)
                            dst = fT[:, ks, it * 128:(it + 1) * 128]
                            nc.scalar.copy(dst, pst)

            def norm_prep(jc):
                """norm^2 = diag(T_jt^T T_jt) via small Gram matmuls, diag
                extracted by identity-mask STT w/ accum; then ONE batched
                sqrt/recip/mul per chunk. Emitted right after the casts so
                beta/cthr are ready long before phase B consumes them."""
                for b in range(JPC):
                    jt = jc * JPC + b
                    gp = gpsum.tile([128, 128], f32, tag="gp",
                                    name=f"gp_{jt}")
                    jb = slice(jt * 128, (jt + 1) * 128)
                    for k2 in range(KS // 2):
                        ksl = slice(2 * k2, 2 * k2 + 2)
                        nc.tensor.matmul(
                            gp, tnc[:, ksl, jb], tnc[:, ksl, jb],
                            perf_mode=DR,
                            start=(k2 == 0), stop=(k2 == KS // 2 - 1),
                        )
                    gsc = scrp.tile([128, 128], bf16, tag="gsc",
                                    name=f"gsc_{jt}")
                    nc.vector.scalar_tensor_tensor(
                        out=gsc, in0=gp, scalar=1.0, in1=ident,
                        op0=OP.mult, op1=OP.mult,
                        accum_out=nrmT[:, jt:jt + 1])
                # nrm64 = 64*||t_j||; beta = 1/nrm64; cthr = margin*nrm64
                jsl = slice(jc * JPC, (jc + 1) * JPC)
                nc.scalar.activation(cthr[:, jsl], nrmT[:, jsl],
                                     AF.Sqrt, scale=4096.0)
                nc.vector.reciprocal(beta[:, jsl], cthr[:, jsl])
                nc.vector.tensor_scalar_mul(cthr[:, jsl], cthr[:, jsl],
                                            MARGIN)

            def phase_B(jc):
                for b in range(JPC):
                    jt = jc * JPC + b
                    jsl1 = slice(jt, jt + 1)
                    for sim, fT in enumerate((f0T, f1T)):
                        ps = mpsum.tile([128, ROWS], f32, tag="ps",
                                        name=f"ps_{jt}_{sim}")
                        for k2 in range(KS // 2):
                            ksl = slice(2 * k2, 2 * k2 + 2)
                            nc.tensor.matmul(
                                ps,
                                tnc[:, ksl, jt * 128:(jt + 1) * 128],
                                fT[:, ksl, :],
                                perf_mode=DR,
                                start=(k2 == 0), stop=(k2 == KS // 2 - 1),
                            )
                        slot = jt * 2 + sim
                        scr = scrp.tile([128, ROWS], bf16, tag="scr",
                                        name=f"scr_{slot}")
                        nc.vector._custom_dve(
                            fused_op,
                            out=scr,
                            in0=ps,
                            in1=LtT[:, jt, :],
                            s0=beta[:, jsl1],
                            s1=cthr[:, jsl1],
                            imm2=0.0,
                            accum_out=qacc[:, slot:slot + 1],
                        )

            # emission order keeps each in-order engine stream aligned
            # with data arrival: prep(0), f, B(0), prep(1), B(1), ...
            chunk_prep(0)
            norm_prep(0)
            f_path()
            for jc in range(JC):
                phase_B(jc)
                if jc + 1 < JC:
                    chunk_prep(jc + 1)
                    norm_prep(jc + 1)

            # ---- finisher ----
            ov = small.tile([128, 1], f32, tag="fin", name="ov")
            nc.vector.reduce_sum(ov, qacc, axis=AX.X)
            nc.sync.dma_start(out_d, ov)

    nc.compile()
    return nc


def _get_nc():
    if "nc" not in _CACHE:
        _CACHE["nc"] = _build_nc()
    return _CACHE["nc"]


def _make_in_maps(fc_feats_0, fc_feats_1, textual_features, labels):
    txT = np.ascontiguousarray(
        np.asarray(textual_features, dtype=np.float32).T)
    labels = np.asarray(labels, dtype=np.float32)
    in_maps = []
    for c in range(NCORES):
        sl = slice(c * ROWS, (c + 1) * ROWS)
        in_maps.append({
            "f0": np.ascontiguousarray(fc_feats_0[sl], dtype=np.float32),
            "f1": np.ascontiguousarray(fc_feats_1[sl], dtype=np.float32),
            "txT": txT,
            "labT": np.ascontiguousarray(labels[sl].T),
        })
    return in_maps


def run(fc_feats_0, fc_feats_1, textual_features, labels, trace=False):
    """Run on 8 NeuronCores; returns (loss_scalar, BassKernelResults)."""
    _import_concourse()
    from concourse.bass_utils import run_bass_kernel_spmd

    nc = _get_nc()
    in_maps = _make_in_maps(np.asarray(fc_feats_0), np.asarray(fc_feats_1),
                            np.asarray(textual_features), np.asarray(labels))
    res = run_bass_kernel_spmd(nc, in_maps, list(range(NCORES)), trace=trace)
    total = 0.0
    for c in range(NCORES):
        total += float(np.asarray(res.results[c]["outv"],
                                  dtype=np.float64).sum())
    loss = total / float(B * B)
    return np.asarray(loss, dtype=np.float32), res


def kernel(fc_feats_0, fc_feats_1, textual_features, labels):
    loss, _ = run(fc_feats_0, fc_feats_1, textual_features, labels,
                  trace=False)
    return loss


# revision 57
# speedup vs baseline: 1.2749x; 1.0246x over previous
"""Trainium2 Bass kernel for nn_ContrastiveLoss (B=4096, D=1024, 8 cores).

loss = mean over [B,B] of, for each sim in (sim0, sim1):
    L*(1-sim) + (1-L)*relu(sim-0.5)
where sim_k = cos_sim(fc_feats_k[i], textual_features[j]).

Per-sim element identity (r = relu(s - m)):
    L*(1-s) + (1-L)*r  =  L + r - L*(s + r)

Strategy (data-parallel i-rows; t replicated -- measured AllGather
latency on this fabric is ~130us for 4MB, far too slow to use):
  * Each core gets its 512-row slice of fc_feats_0/1, its labels slice
    TRANSPOSED on host ([B, 512], pure layout), and the FULL
    textual_features transposed on host ([D, B], pure layout).
  * Phase B matmuls put t on the STATIONARY side: out psum is
    [128 j-part, 512 i-free], so the per-j t-norm is a per-PARTITION
    scalar -- no elementwise renormalization pass over t at all.
    t is cast raw to fp8 (values ~N(0,1), in e4m3 range).
  * t column norms: ACT squares (fp8) + fp8 ones-matvec accumulated
    over d in PSUM [1,512], sqrt, tiny PE transposes into [j-part,1]
    vectors, then beta = 1/(64*||t_j||) and c = 32*||t_j|| as [128,1]
    scalars per j-tile.
  * One custom DVE instruction per psum tile computes (u = raw psum):
        q = beta_j*(ru - L*(u + ru)) + L,   ru = relu(u - c_j)
    with free-dim accumulation; summing q over everything equals the
    sum of loss elements exactly (s = beta*u, r = beta*ru).
  * f0/f1: row-normalized (x64) to fp8 and PE-transposed as usual;
    they are the MOVING operand [d, 512 i].
  * DMA: sync queue (engine has no compute, may block freely) carries
    all 16MB of t; the gpsimd SW queue carries f0/f1 then labels (f32).
    Compute engines never issue DMAs -- a blocked DMA issue stalls the
    whole engine stream.
  * Host sums the 8x[128] partials and divides by B*B.

Self-contained: hardcodes shapes; only needs the concourse package.
"""

import os
import sys

import numpy as np

B = 4096
D = 1024
NCORES = 8
ROWS = B // NCORES          # 512 i-rows of f0/f1 per core
IT = ROWS // 128            # 4 f i-tiles per core
KS = D // 128               # 8 k-subtiles (contraction)
JTILES = B // 128           # 32 j-tiles
MARGIN = 0.5

JC = int(os.environ.get("KERNEL_TCHUNKS", "4"))  # t j-chunks
CW = B // JC                # columns per chunk (1024)
WPC = CW // 512             # 512-wide matvec windows per chunk
JPC = CW // 128             # j-tiles per chunk (8)
NSLOT = JTILES * 2          # accumulation slots (x2 sims) = 64

_CACHE = {}


def _import_concourse():
    try:
        import concourse.bass  # noqa: F401
    except ImportError:
        for p in ("/opt/trn_rl_repo", "/root/.axon_site/_ro/trn_rl_repo"):
            if os.path.isdir(p) and p not in sys.path:
                sys.path.insert(0, p)
        import concourse.bass  # noqa: F401


def _register_fused_op():
    """Register the fused contrastive-loss DVE op (idempotent).

    body = (r - Src1*(Src0 + r))*C0 + Src1, r = relu(Src0 - C1); accum=add.
    s0 (C0) and s1 (C1) are per-partition [128,1] APs at the call site.
    """
    from operator import add

    import concourse.dve_ops as dve_ops
    from concourse.dve_spec import C0, C1, Spec, Src0, Src1, Zero, lower, relu
    from concourse.dve_uop import DveOpSpec

    name = "CL_FUSED_Q_ANT"
    if name in dve_ops._SUB_OPCODE_FOR_NAME:
        return next(op for op in dve_ops.OPS if op.name == name)

    r_ = relu(Src0 - C1)
    body = (r_ - Src1 * (Src0 + r_)) * C0 + Src1

    def _ref(in0, in1, c0, c1, c2):
        x = in0.astype(np.float32)
        L = in1.astype(np.float32)
        r = np.maximum(np.nan_to_num(x - c1, nan=0.0), 0.0)
        b = ((r - L * (x + r)) * c0 + L).astype(np.float32)
        return b, b.reshape(b.shape[0], -1).sum(axis=-1, keepdims=True)

    spec = Spec(body=body, accum=add, accum_init=Zero, reference=_ref)
    row = max(dve_ops._SUB_OPCODE_FOR_NAME.values()) + 1
    assert row < 0x20
    dve_ops._SUB_OPCODE_FOR_NAME[name] = row
    shas = {}
    for ver in ("v3", "v4"):
        try:
            uops = lower(spec, ver=ver)
        except Exception:
            continue
        shas[ver] = DveOpSpec(
            name=name, opcode=row, uops=uops, rd1_en=True).sha(ver)
    op = dve_ops.DveOp(name, spec, False, shas)
    dve_ops.OPS.append(op)
    dve_ops.CUSTOM_DVE_SPECS[name] = spec
    return op


def _build_nc():
    """Build + schedule + compile the per-core Bass program (SPMD)."""
    _import_concourse()
    import concourse.mybir as mybir
    import concourse.tile as tile
    from concourse import bacc
    from concourse.masks import make_identity

    fused_op = _register_fused_op()

    f32 = mybir.dt.float32
    bf16 = mybir.dt.bfloat16
    wdt = mybir.dt.float8e4
    AF = mybir.ActivationFunctionType
    OP = mybir.AluOpType
    AX = mybir.AxisListType
    DR = mybir.MatmulPerfMode.DoubleRow

    nc = bacc.Bacc(
        "TRN2",
        target_bir_lowering=False,
        debug=False,
        num_devices=NCORES,
    )

    f0_d = nc.dram_tensor("f0", [ROWS, D], f32, kind="ExternalInput").ap()
    f1_d = nc.dram_tensor("f1", [ROWS, D], f32, kind="ExternalInput").ap()
    txT_d = nc.dram_tensor("txT", [D, B], f32, kind="ExternalInput").ap()
    labT_d = nc.dram_tensor("labT", [B, ROWS], f32,
                            kind="ExternalInput").ap()
    out_d = nc.dram_tensor("outv", [128, 1], f32, kind="ExternalOutput").ap()

    with tile.TileContext(nc) as tc:
        with (
            tc.tile_pool(name="constp", bufs=1) as constp,
            tc.tile_pool(name="traw", bufs=12) as trawp,
            tc.tile_pool(name="fnat", bufs=4) as fnatp,
            tc.tile_pool(name="stage", bufs=3) as stage,
            tc.tile_pool(name="small", bufs=8) as small,
            tc.tile_pool(name="nrm", bufs=1) as nrmp,
            tc.tile_pool(name="wT", bufs=1) as wTp,
            tc.tile_pool(name="tnp", bufs=1) as tnp,
            tc.tile_pool(name="labp", bufs=1) as labp,
            tc.tile_pool(name="scrp", bufs=2) as scrp,
            tc.tile_pool(name="accp", bufs=1) as accp,
            tc.tile_pool(name="tpsum", bufs=2, space="PSUM") as tpsum,
            tc.tile_pool(name="gpsum", bufs=1, space="PSUM") as gpsum,
            tc.tile_pool(name="mpsum", bufs=5, space="PSUM") as mpsum,
        ):
            ident = constp.tile([128, 128], bf16)
            make_identity(nc, ident)

            qacc = accp.tile([128, NSLOT], f32)

            f0T = wTp.tile([128, KS, ROWS], wdt)   # moving operands (x64)
            f1T = wTp.tile([128, KS, ROWS], wdt)
            tnc = tnp.tile([128, KS, B], wdt)      # raw fp8 t (stationary)
            LtT = labp.tile([128, JTILES, ROWS], f32)    # labels^T
            # per-j-tile norm-derived scalars
            nrmT = nrmp.tile([128, JTILES], f32)   # ||t_j||
            beta = nrmp.tile([128, JTILES], f32)   # 1/(64 ||t_j||)
            cthr = nrmp.tile([128, JTILES], f32)   # 64*MARGIN*||t_j||

            # ---- input DMAs on parallel queues ----
            # sync queue: t even-ks halves interleaved with label groups
            # scalar queue: chunk0 odd half, f0/f1, remaining odd halves
            labT3 = labT_d.rearrange("(g p) i -> p g i", p=128)
            traws = {}
            # sync queue (no compute on that engine): ALL t tiles
            for jc in range(JC):
                for ks in range(KS):
                    tr = trawp.tile([128, CW], f32, tag="traw",
                                    name=f"traw_{jc}_{ks}")
                    nc.sync.dma_start(
                        tr, txT_d[ks * 128:(ks + 1) * 128,
                                  jc * CW:(jc + 1) * CW])
                    traws[(jc, ks)] = tr
            # gpsimd SW queue: f first, then labels (f32)
            fnats = []
            for fi, src in enumerate((f0_d, f1_d)):
                for it in range(IT):
                    nat = fnatp.tile([128, D], f32, tag="fnat",
                                     name=f"fnat_{fi}_{it}")
                    nc.gpsimd.dma_start(nat, src[it * 128:(it + 1) * 128, :])
                    fnats.append(nat)
            for jg in range(4):
                nc.gpsimd.dma_start(
                    LtT[:, jg * 8:(jg + 1) * 8, :],
                    labT3[:, jg * 8:(jg + 1) * 8, :],
                )

            def chunk_prep(jc):
                """fp8 casts for t-chunk jc (ACT only; no DMA issuers)."""
                c0 = jc * CW
                for ks in range(KS):
                    nc.scalar.copy(tnc[:, ks, c0:c0 + CW], traws[(jc, ks)])

            def f_path():
                for it in range(IT):
                    for fi, fT in enumerate((f0T, f1T)):
                        nat = fnats[fi * IT + it]
                        key = f"f{fi}_{it}"
                        sq = stage.tile([128, D], bf16, tag="fsq",
                                        name=f"fsq_{key}")
                        ssq = small.tile([128, 1], f32, tag="ssq",
                                         name=f"ssq_{key}")
                        nc.scalar.activation(sq, nat, AF.Square,
                                             accum_out=ssq)
                        nrm = small.tile([128, 1], f32, tag="nrm",
                                         name=f"nrm_{key}")
                        nc.scalar.activation(nrm, ssq, AF.Sqrt,
                                             scale=1.0 / 4096.0)
                        rin = small.tile([128, 1], f32, tag="rin",
                                         name=f"rin_{key}")
                        nc.vector.reciprocal(rin, nrm)   # 64 / ||f_i||
                        nrmd = stage.tile([128, D], bf16, tag="nrmd",
                                          name=f"nrmd_{key}")
                        nc.vector.tensor_scalar_mul(nrmd, nat, rin)
                        for ks in range(KS):
                            pst = tpsum.tile([128, 128], bf16, tag="pst",
                                             name=f"pst_{key}_{ks}")
                            nc.tensor.transpose(
                                pst, nrmd[:, ks * 128:(ks + 1) * 128], ident)
                            dst = fT[:, ks, it * 128:(it + 1) * 128]
                            nc.scalar.copy(dst, pst)

            def phase_B(jc):
                for b in range(JPC):
                    jt = jc * JPC + b
                    jsl1 = slice(jt, jt + 1)
                    # norm^2 = diag(T_jt^T T_jt) via a small Gram matmul,
                    # diag extracted by a gpsimd identity-mask STT w/ accum
                    gp = gpsum.tile([128, 128], f32, tag="gp",
                                    name=f"gp_{jt}")
                    jb = slice(jt * 128, (jt + 1) * 128)
                    for k2 in range(KS // 2):
                        ksl = slice(2 * k2, 2 * k2 + 2)
                        nc.tensor.matmul(
                            gp, tnc[:, ksl, jb], tnc[:, ksl, jb],
                            perf_mode=DR,
                            start=(k2 == 0), stop=(k2 == KS // 2 - 1),
                        )
                    gsc = scrp.tile([128, 128], bf16, tag="gsc",
                                    name=f"gsc_{jt}")
                    nc.vector.scalar_tensor_tensor(
                        out=gsc, in0=gp, scalar=1.0, in1=ident,
                        op0=OP.mult, op1=OP.mult,
                        accum_out=nrmT[:, jsl1])
                    # nrm64 = 64*||t_j||; beta = 1/nrm64; cthr = margin*nrm64
                    nc.scalar.activation(cthr[:, jsl1], nrmT[:, jsl1],
                                         AF.Sqrt, scale=4096.0)
                    nc.vector.reciprocal(beta[:, jsl1], cthr[:, jsl1])
                    nc.vector.tensor_scalar_mul(cthr[:, jsl1],
                                                cthr[:, jsl1], MARGIN)
                    for sim, fT in enumerate((f0T, f1T)):
                        ps = mpsum.tile([128, ROWS], f32, tag="ps",
                                        name=f"ps_{jt}_{sim}")
                        for k2 in range(KS // 2):
                            ksl = slice(2 * k2, 2 * k2 + 2)
                            nc.tensor.matmul(
                                ps,
                                tnc[:, ksl, jt * 128:(jt + 1) * 128],
                                fT[:, ksl, :],
                                perf_mode=DR,
                                start=(k2 == 0), stop=(k2 == KS // 2 - 1),
                            )
                        slot = jt * 2 + sim
                        scr = scrp.tile([128, ROWS], bf16, tag="scr",
                                        name=f"scr_{slot}")
                        nc.vector._custom_dve(
                            fused_op,
                            out=scr,
                            in0=ps,
                            in1=LtT[:, jt, :],
                            s0=beta[:, jsl1],
                            s1=cthr[:, jsl1],
                            imm2=0.0,
                            accum_out=qacc[:, slot:slot + 1],
                        )

            # emission order keeps each in-order engine stream aligned
            # with data arrival: prep(0), f, B(0), prep(1), B(1), ...
            chunk_prep(0)
            f_path()
            for jc in range(JC):
                phase_B(jc)
                if jc + 1 < JC:
                    chunk_prep(jc + 1)

            # ---- finisher ----
            ov = small.tile([128, 1], f32, tag="fin", name="ov")
            nc.vector.reduce_sum(ov, qacc, axis=AX.X)
            nc.sync.dma_start(out_d, ov)

    nc.compile()
    return nc


def _get_nc():
    if "nc" not in _CACHE:
        _CACHE["nc"] = _build_nc()
    return _CACHE["nc"]


def _make_in_maps(fc_feats_0, fc_feats_1, textual_features, labels):
    txT = np.ascontiguousarray(
        np.asarray(textual_features, dtype=np.float32).T)
    labels = np.asarray(labels, dtype=np.float32)
    in_maps = []
    for c in range(NCORES):
        sl = slice(c * ROWS, (c + 1) * ROWS)
        in_maps.append({
            "f0": np.ascontiguousarray(fc_feats_0[sl], dtype=np.float32),
            "f1": np.ascontiguousarray(fc_feats_1[sl], dtype=np.float32),
            "txT": txT,
            "labT": np.ascontiguousarray(labels[sl].T),
        })
    return in_maps


def run(fc_feats_0, fc_feats_1, textual_features, labels, trace=False):
    """Run on 8 NeuronCores; returns (loss_scalar, BassKernelResults)."""
    _import_concourse()
    from concourse.bass_utils import run_bass_kernel_spmd

    nc = _get_nc()
    in_maps = _make_in_maps(np.asarray(fc_feats_0), np.asarray(fc_feats_1),
                            np.asarray(textual_features), np.asarray(labels))
    res = run_bass_kernel_spmd(nc, in_maps, list(range(NCORES)), trace=trace)
    total = 0.0
    for c in range(NCORES):
        total += float(np.asarray(res.results[c]["outv"],
                                  dtype=np.float64).sum())
    loss = total / float(B * B)
    return np.asarray(loss, dtype=np.float32), res


def kernel(fc_feats_0, fc_feats_1, textual_features, labels):
    loss, _ = run(fc_feats_0, fc_feats_1, textual_features, labels,
                  trace=False)
    return loss


# revision 58
# speedup vs baseline: 1.2959x; 1.0165x over previous
"""Trainium2 Bass kernel for nn_ContrastiveLoss (B=4096, D=1024, 8 cores).

loss = mean over [B,B] of, for each sim in (sim0, sim1):
    L*(1-sim) + (1-L)*relu(sim-0.5)
where sim_k = cos_sim(fc_feats_k[i], textual_features[j]).

Per-sim element identity (r = relu(s - m)):
    L*(1-s) + (1-L)*r  =  L + r - L*(s + r)

Strategy (data-parallel i-rows; t replicated -- measured AllGather
latency on this fabric is ~130us for 4MB, far too slow to use):
  * Each core gets its 512-row slice of fc_feats_0/1, its labels slice
    TRANSPOSED on host ([B, 512], pure layout), and the FULL
    textual_features transposed on host ([D, B], pure layout).
  * Phase B matmuls put t on the STATIONARY side: out psum is
    [128 j-part, 512 i-free], so the per-j t-norm is a per-PARTITION
    scalar -- no elementwise renormalization pass over t at all.
    t is cast raw to fp8 (values ~N(0,1), in e4m3 range).
  * t column norms: ACT squares (fp8) + fp8 ones-matvec accumulated
    over d in PSUM [1,512], sqrt, tiny PE transposes into [j-part,1]
    vectors, then beta = 1/(64*||t_j||) and c = 32*||t_j|| as [128,1]
    scalars per j-tile.
  * One custom DVE instruction per psum tile computes (u = raw psum):
        q = beta_j*(ru - L*(u + ru)) + L,   ru = relu(u - c_j)
    with free-dim accumulation; summing q over everything equals the
    sum of loss elements exactly (s = beta*u, r = beta*ru).
  * f0/f1: row-normalized (x64) to fp8 and PE-transposed as usual;
    they are the MOVING operand [d, 512 i].
  * DMA: sync queue (engine has no compute, may block freely) carries
    all 16MB of t; the gpsimd SW queue carries f0/f1 then labels (f32).
    Compute engines never issue DMAs -- a blocked DMA issue stalls the
    whole engine stream.
  * Host sums the 8x[128] partials and divides by B*B.

Self-contained: hardcodes shapes; only needs the concourse package.
"""

import os
import sys

import numpy as np

B = 4096
D = 1024
NCORES = 8
ROWS = B // NCORES          # 512 i-rows of f0/f1 per core
IT = ROWS // 128            # 4 f i-tiles per core
KS = D // 128               # 8 k-subtiles (contraction)
JTILES = B // 128           # 32 j-tiles
MARGIN = 0.5

JC = int(os.environ.get("KERNEL_TCHUNKS", "4"))  # t j-chunks
CW = B // JC                # columns per chunk (1024)
WPC = CW // 512             # 512-wide matvec windows per chunk
JPC = CW // 128             # j-tiles per chunk (8)
NSLOT = JTILES * 2          # accumulation slots (x2 sims) = 64

_CACHE = {}


def _import_concourse():
    try:
        import concourse.bass  # noqa: F401
    except ImportError:
        for p in ("/opt/trn_rl_repo", "/root/.axon_site/_ro/trn_rl_repo"):
            if os.path.isdir(p) and p not in sys.path:
                sys.path.insert(0, p)
        import concourse.bass  # noqa: F401


def _register_fused_op():
    """Register the fused contrastive-loss DVE op (idempotent).

    body = (r - Src1*(Src0 + r))*C0 + Src1, r = relu(Src0 - C1); accum=add.
    s0 (C0) and s1 (C1) are per-partition [128,1] APs at the call site.
    """
    from operator import add

    import concourse.dve_ops as dve_ops
    from concourse.dve_spec import C0, C1, Spec, Src0, Src1, Zero, lower, relu
    from concourse.dve_uop import DveOpSpec

    name = "CL_FUSED_Q_ANT"
    if name in dve_ops._SUB_OPCODE_FOR_NAME:
        return next(op for op in dve_ops.OPS if op.name == name)

    r_ = relu(Src0 - C1)
    body = (r_ - Src1 * (Src0 + r_)) * C0 + Src1

    def _ref(in0, in1, c0, c1, c2):
        x = in0.astype(np.float32)
        L = in1.astype(np.float32)
        r = np.maximum(np.nan_to_num(x - c1, nan=0.0), 0.0)
        b = ((r - L * (x + r)) * c0 + L).astype(np.float32)
        return b, b.reshape(b.shape[0], -1).sum(axis=-1, keepdims=True)

    spec = Spec(body=body, accum=add, accum_init=Zero, reference=_ref)
    row = max(dve_ops._SUB_OPCODE_FOR_NAME.values()) + 1
    assert row < 0x20
    dve_ops._SUB_OPCODE_FOR_NAME[name] = row
    shas = {}
    for ver in ("v3", "v4"):
        try:
            uops = lower(spec, ver=ver)
        except Exception:
            continue
        shas[ver] = DveOpSpec(
            name=name, opcode=row, uops=uops, rd1_en=True).sha(ver)
    op = dve_ops.DveOp(name, spec, False, shas)
    dve_ops.OPS.append(op)
    dve_ops.CUSTOM_DVE_SPECS[name] = spec
    return op


def _build_nc():
    """Build + schedule + compile the per-core Bass program (SPMD)."""
    _import_concourse()
    import concourse.mybir as mybir
    import concourse.tile as tile
    from concourse import bacc
    from concourse.masks import make_identity

    fused_op = _register_fused_op()

    f32 = mybir.dt.float32
    bf16 = mybir.dt.bfloat16
    wdt = mybir.dt.float8e4
    AF = mybir.ActivationFunctionType
    OP = mybir.AluOpType
    AX = mybir.AxisListType
    DR = mybir.MatmulPerfMode.DoubleRow

    nc = bacc.Bacc(
        "TRN2",
        target_bir_lowering=False,
        debug=False,
        num_devices=NCORES,
    )

    f0_d = nc.dram_tensor("f0", [ROWS, D], f32, kind="ExternalInput").ap()
    f1_d = nc.dram_tensor("f1", [ROWS, D], f32, kind="ExternalInput").ap()
    txT_d = nc.dram_tensor("txT", [D, B], f32, kind="ExternalInput").ap()
    labT_d = nc.dram_tensor("labT", [B, ROWS], f32,
                            kind="ExternalInput").ap()
    out_d = nc.dram_tensor("outv", [128, 1], f32, kind="ExternalOutput").ap()

    with tile.TileContext(nc) as tc:
        with (
            tc.tile_pool(name="constp", bufs=1) as constp,
            tc.tile_pool(name="traw", bufs=12) as trawp,
            tc.tile_pool(name="fnat", bufs=4) as fnatp,
            tc.tile_pool(name="stage", bufs=3) as stage,
            tc.tile_pool(name="small", bufs=8) as small,
            tc.tile_pool(name="nrm", bufs=1) as nrmp,
            tc.tile_pool(name="wT", bufs=1) as wTp,
            tc.tile_pool(name="tnp", bufs=1) as tnp,
            tc.tile_pool(name="labp", bufs=1) as labp,
            tc.tile_pool(name="scrp", bufs=2) as scrp,
            tc.tile_pool(name="accp", bufs=1) as accp,
            tc.tile_pool(name="tpsum", bufs=2, space="PSUM") as tpsum,
            tc.tile_pool(name="gpsum", bufs=2, space="PSUM") as gpsum,
            tc.tile_pool(name="mpsum", bufs=4, space="PSUM") as mpsum,
        ):
            ident = constp.tile([128, 128], bf16)
            make_identity(nc, ident)

            qacc = accp.tile([128, NSLOT], f32)

            f0T = wTp.tile([128, KS, ROWS], wdt)   # moving operands (x64)
            f1T = wTp.tile([128, KS, ROWS], wdt)
            tnc = tnp.tile([128, KS, B], wdt)      # raw fp8 t (stationary)
            LtT = labp.tile([128, JTILES, ROWS], f32)    # labels^T
            # per-j-tile norm-derived scalars
            nrmT = nrmp.tile([128, JTILES], f32)   # ||t_j||
            beta = nrmp.tile([128, JTILES], f32)   # 1/(64 ||t_j||)
            cthr = nrmp.tile([128, JTILES], f32)   # 64*MARGIN*||t_j||

            # ---- input DMAs on parallel queues ----
            # sync queue: t even-ks halves interleaved with label groups
            # scalar queue: chunk0 odd half, f0/f1, remaining odd halves
            labT3 = labT_d.rearrange("(g p) i -> p g i", p=128)
            traws = {}
            # sync queue (no compute on that engine): ALL t tiles
            for jc in range(JC):
                for ks in range(KS):
                    tr = trawp.tile([128, CW], f32, tag="traw",
                                    name=f"traw_{jc}_{ks}")
                    nc.sync.dma_start(
                        tr, txT_d[ks * 128:(ks + 1) * 128,
                                  jc * CW:(jc + 1) * CW])
                    traws[(jc, ks)] = tr
            # gpsimd SW queue: f first, then labels (f32)
            fnats = []
            for fi, src in enumerate((f0_d, f1_d)):
                for it in range(IT):
                    nat = fnatp.tile([128, D], f32, tag="fnat",
                                     name=f"fnat_{fi}_{it}")
                    nc.gpsimd.dma_start(nat, src[it * 128:(it + 1) * 128, :])
                    fnats.append(nat)
            for jg in range(4):
                nc.gpsimd.dma_start(
                    LtT[:, jg * 8:(jg + 1) * 8, :],
                    labT3[:, jg * 8:(jg + 1) * 8, :],
                )

            def chunk_prep(jc):
                """fp8 casts for t-chunk jc (ACT only; no DMA issuers)."""
                c0 = jc * CW
                for ks in range(KS):
                    nc.scalar.copy(tnc[:, ks, c0:c0 + CW], traws[(jc, ks)])

            def f_path():
                for it in range(IT):
                    for fi, fT in enumerate((f0T, f1T)):
                        nat = fnats[fi * IT + it]
                        key = f"f{fi}_{it}"
                        sq = stage.tile([128, D], bf16, tag="fsq",
                                        name=f"fsq_{key}")
                        ssq = small.tile([128, 1], f32, tag="ssq",
                                         name=f"ssq_{key}")
                        nc.scalar.activation(sq, nat, AF.Square,
                                             accum_out=ssq)
                        nrm = small.tile([128, 1], f32, tag="nrm",
                                         name=f"nrm_{key}")
                        nc.scalar.activation(nrm, ssq, AF.Sqrt,
                                             scale=1.0 / 4096.0)
                        rin = small.tile([128, 1], f32, tag="rin",
                                         name=f"rin_{key}")
                        nc.vector.reciprocal(rin, nrm)   # 64 / ||f_i||
                        nrmd = stage.tile([128, D], bf16, tag="nrmd",
                                          name=f"nrmd_{key}")
                        nc.vector.tensor_scalar_mul(nrmd, nat, rin)
                        for ks in range(KS):
                            pst = tpsum.tile([128, 128], bf16, tag="pst",
                                             name=f"pst_{key}_{ks}")
                            nc.tensor.transpose(
                                pst, nrmd[:, ks * 128:(ks + 1) * 128], ident)
                            dst = fT[:, ks, it * 128:(it + 1) * 128]
                            nc.scalar.copy(dst, pst)

            def phase_B(jc):
                for b in range(JPC):
                    jt = jc * JPC + b
                    jsl1 = slice(jt, jt + 1)
                    # norm^2 = diag(T_jt^T T_jt) via a small Gram matmul,
                    # diag extracted by a gpsimd identity-mask STT w/ accum
                    gp = gpsum.tile([128, 128], f32, tag="gp",
                                    name=f"gp_{jt}")
                    jb = slice(jt * 128, (jt + 1) * 128)
                    for k2 in range(KS // 2):
                        ksl = slice(2 * k2, 2 * k2 + 2)
                        nc.tensor.matmul(
                            gp, tnc[:, ksl, jb], tnc[:, ksl, jb],
                            perf_mode=DR,
                            start=(k2 == 0), stop=(k2 == KS // 2 - 1),
                        )
                    gsc = scrp.tile([128, 128], bf16, tag="gsc",
                                    name=f"gsc_{jt}")
                    nc.vector.scalar_tensor_tensor(
                        out=gsc, in0=gp, scalar=1.0, in1=ident,
                        op0=OP.mult, op1=OP.mult,
                        accum_out=nrmT[:, jsl1])
                    # nrm64 = 64*||t_j||; beta = 1/nrm64; cthr = margin*nrm64
                    nc.scalar.activation(cthr[:, jsl1], nrmT[:, jsl1],
                                         AF.Sqrt, scale=4096.0)
                    nc.vector.reciprocal(beta[:, jsl1], cthr[:, jsl1])
                    nc.vector.tensor_scalar_mul(cthr[:, jsl1],
                                                cthr[:, jsl1], MARGIN)
                    for sim, fT in enumerate((f0T, f1T)):
                        ps = mpsum.tile([128, ROWS], f32, tag="ps",
                                        name=f"ps_{jt}_{sim}")
                        for k2 in range(KS // 2):
                            ksl = slice(2 * k2, 2 * k2 + 2)
                            nc.tensor.matmul(
                                ps,
                                tnc[:, ksl, jt * 128:(jt + 1) * 128],
                                fT[:, ksl, :],
                                perf_mode=DR,
                                start=(k2 == 0), stop=(k2 == KS // 2 - 1),
                            )
                        slot = jt * 2 + sim
                        scr = scrp.tile([128, ROWS], bf16, tag="scr",
                                        name=f"scr_{slot}")
                        nc.vector._custom_dve(
                            fused_op,
                            out=scr,
                            in0=ps,
                            in1=LtT[:, jt, :],
                            s0=beta[:, jsl1],
                            s1=cthr[:, jsl1],
                            imm2=0.0,
                            accum_out=qacc[:, slot:slot + 1],
                        )

            # emission order keeps each in-order engine stream aligned
            # with data arrival: prep(0), f, B(0), prep(1), B(1), ...
            chunk_prep(0)
            f_path()
            for jc in range(JC):
                phase_B(jc)
                if jc + 1 < JC:
                    chunk_prep(jc + 1)

            # ---- finisher ----
            ov = small.tile([128, 1], f32, tag="fin", name="ov")
            nc.vector.reduce_sum(ov, qacc, axis=AX.X)
            nc.sync.dma_start(out_d, ov)

    nc.compile()
    return nc


def _get_nc():
    if "nc" not in _CACHE:
        _CACHE["nc"] = _build_nc()
    return _CACHE["nc"]


def _make_in_maps(fc_feats_0, fc_feats_1, textual_features, labels):
    txT = np.ascontiguousarray(
        np.asarray(textual_features, dtype=np.float32).T)
    labels = np.asarray(labels, dtype=np.float32)
    in_maps = []
    for c in range(NCORES):
        sl = slice(c * ROWS, (c + 1) * ROWS)
        in_maps.append({
            "f0": np.ascontiguousarray(fc_feats_0[sl], dtype=np.float32),
            "f1": np.ascontiguousarray(fc_feats_1[sl], dtype=np.float32),
            "txT": txT,
            "labT": np.ascontiguousarray(labels[sl].T),
        })
    return in_maps


def run(fc_feats_0, fc_feats_1, textual_features, labels, trace=False):
    """Run on 8 NeuronCores; returns (loss_scalar, BassKernelResults)."""
    _import_concourse()
    from concourse.bass_utils import run_bass_kernel_spmd

    nc = _get_nc()
    in_maps = _make_in_maps(np.asarray(fc_feats_0), np.asarray(fc_feats_1),
                            np.asarray(textual_features), np.asarray(labels))
    res = run_bass_kernel_spmd(nc, in_maps, list(range(NCORES)), trace=trace)
    total = 0.0
    for c in range(NCORES):
        total += float(np.asarray(res.results[c]["outv"],
                                  dtype=np.float64).sum())
    loss = total / float(B * B)
    return np.asarray(loss, dtype=np.float32), res


def kernel(fc_feats_0, fc_feats_1, textual_features, labels):
    loss, _ = run(fc_feats_0, fc_feats_1, textual_features, labels,
                  trace=False)
    return loss
